# revision 39
# baseline (speedup 1.0000x reference)
"""Trainium2 Bass kernel for packed-sequence GQA attention (nn_Attention_84602265796942).

Sharding: data-parallel over the B=16 packed sequences -> 2 sequences (1024
tokens) per core, weights replicated. Zero collectives.

Per-core pipeline (all matmuls bf16 x bf16 -> fp32 PSUM):
  A) QKV projections from host-transposed inputs; RoPE applied via a
     host-side head-dim permutation ([a0..a15 b0..b15 a16..a31 b16..b31])
     so the rotation partner sits at partition r^16 (one stream_shuffle).
  B) Attention per (block, head-pair), emitted interleaved with the Q
     projection so PE/ACT/DVE overlap across phases: q-heads are
     host-permuted so pair (h, h+4) shares a qt tile and maps to kv heads
     (2g, 2g+1) = the two partition halves of one K tile; scores are
     computed transposed (scoresT[m,l]) with paired K=64 matmuls at
     partition bases 0/64 (concurrent PE row groups); softmax without max
     subtraction (scores are bounded); probs row-sums obtained via a
     ones-column appended to V in the PV matmul (M=65); normalization
     deferred to a per-column scale of the attention output.
  C) Output projection from the transposed attention output.

PSUM pools are sized 4+2+2 banks so projection, score, and PV
accumulations coexist within the 8 banks and the phases pipeline; every
PSUM tile has a single reader that stages it to SBUF so banks release
as early as possible.
"""
import numpy as np
import ml_dtypes

import concourse.bass as bass
import concourse.tile as tile
from concourse import bacc, mybir
from concourse.bass_utils import run_bass_kernel_spmd

F32 = mybir.dt.float32
BF16 = mybir.dt.bfloat16

B, L, DIM, H, HKV, DH = 16, 512, 2048, 32, 8, 64
REP = H // HKV
S = B * L
NCORE = 8
S_LOC = S // NCORE          # 1024 tokens per core
NBLK = S_LOC // L           # 2 blocks per core
SCALE = DH ** -0.5

# within-head dim permutation: rows [a0..a15, b0..b15, a16..a31, b16..b31]
PERM64 = np.concatenate([np.arange(0, 32, 2), np.arange(1, 32, 2),
                         np.arange(32, 64, 2), np.arange(33, 64, 2)])
_rr = np.arange(64)
FREQ_IDX = (_rr // 32) * 16 + (_rr % 16)
C2_SIGN = np.where((_rr % 32) < 16, -1.0, 1.0).astype(np.float32)
# q-head order: pair (h, h+4) within each group of 8 -> kv heads (2g, 2g+1)
HPERM = np.array([8 * gi + t + 4 * half
                  for gi in range(4) for t in range(4) for half in range(2)])

_CACHED = {}

LAST_RESULTS = None  # BassKernelResults of the most recent run (for test.py)


def _build():
    nc = bacc.Bacc("TRN2", target_bir_lowering=False, debug=False,
                   num_devices=NCORE)

    xT_d = nc.dram_tensor("xT", [DIM, S_LOC], BF16, kind="ExternalInput")
    wqT_d = nc.dram_tensor("wqT", [DIM, H * DH], BF16, kind="ExternalInput")
    wkT_d = nc.dram_tensor("wkT", [DIM, HKV * DH], BF16, kind="ExternalInput")
    wvT_d = nc.dram_tensor("wvT", [DIM, HKV * DH], BF16, kind="ExternalInput")
    woT_d = nc.dram_tensor("woT", [H * DH, DIM], BF16, kind="ExternalInput")
    c1_d = nc.dram_tensor("c1", [128, S_LOC], BF16, kind="ExternalInput")
    c2_d = nc.dram_tensor("c2", [128, S_LOC], BF16, kind="ExternalInput")
    out_d = nc.dram_tensor("out", [S_LOC, DIM], F32, kind="ExternalOutput")

    KD = DIM // 128          # 16 contraction tiles
    NQI = (H * DH) // 128    # 16 Q row-tiles (one head pair each)
    NKI = (HKV * DH) // 128  # 4 K row-tiles
    NMT = L // 128           # 4 token tiles per block
    EXP = mybir.ActivationFunctionType.Exp
    SHUF_MASK = [i ^ 16 for i in range(32)]

    with tile.TileContext(nc) as tc:
        with (
            tc.tile_pool(name="persist", bufs=1) as pp,      # long-lived activations
            tc.tile_pool(name="scratch", bufs=2) as sp,      # rope/norm scratch
            tc.tile_pool(name="wop", bufs=1) as pwo,         # wo, prefetched in phase B
        ):
            # persistent activation tensors (qt is a short-lived ring in sp)
            kt = [[pp.tile([128, L], BF16, tag=f"kt{g}_{b}", name=f"kt{g}_{b}")
                   for b in range(NBLK)] for g in range(NKI)]
            vaug = [pp.tile([128, HKV * (DH + 1)], BF16, tag=f"va{m}", name=f"va{m}")
                    for m in range(S_LOC // 128)]
            att = [[pp.tile([128, L], BF16, tag=f"at{i}_{b}", name=f"at{i}_{b}")
                    for b in range(NBLK)] for i in range(NQI)]
            woT = [None] * NQI

            # ================= phase A: QKV projections + RoPE =================
            with (
                tc.tile_pool(name="inA", bufs=1) as pa,
                tc.tile_pool(name="psA", bufs=3, space="PSUM") as psA,
            ):
                def rope_epilogue(ps, b, dst128):
                    """ps: [128, 512] psum of pre-rope QT/KT rows -> bf16 dst.

                    4 DVE ops: t1 and the shuffle both read PSUM directly
                    (no staging copy; the psA slot frees after the second
                    reader, well within the 4-slot rotation slack)."""
                    cs = slice(b * L, (b + 1) * L)
                    t1 = sp.tile([128, L], BF16, tag="t1", name="t1")
                    nc.vector.tensor_mul(t1[:], ps[:], c1[:, cs])
                    sh = sp.tile([128, L], F32, tag="sh", name="sh")
                    nc.vector.stream_shuffle(sh[:], ps[:], SHUF_MASK)
                    t2 = sp.tile([128, L], BF16, tag="t2", name="t2")
                    nc.vector.tensor_mul(t2[:], sh[:], c2[:, cs])
                    nc.vector.tensor_add(dst128[:], t1[:], t2[:])

                xT = []
                c1 = pa.tile([128, S_LOC], BF16, tag="c1", name="c1s")
                c2 = pa.tile([128, S_LOC], BF16, tag="c2", name="c2s")

                with tc.tile_pool(name="inKV", bufs=1) as pkv:
                    wkT, wvT = [], []
                    # x is loaded in block halves: the block-0 half tiles
                    # (xa) interleave with merged wv chunks so V-proj pass 1
                    # (m 0..3, block-0 tokens) is PE-bound from the first
                    # pair; the block-1 halves (xb) follow as merged chunks
                    xa, xb = [], []
                    k0 = 0
                    for nw, wtiles in enumerate((2, 2, 4, 4, 4)):
                        # wv chunk sizes ramp up: a tiny first chunk
                        # minimizes the latency to the first V matmul
                        wvc = pkv.tile([128, wtiles, HKV * DH], BF16,
                                       tag=f"wvT{nw}", name=f"wvT{nw}")
                        nc.sync.dma_start(
                            wvc[:],
                            wvT_d[k0 * 128:(k0 + wtiles) * 128, :]
                            .rearrange("(j p) c -> p j c", p=128))
                        wvT.extend(wvc[:, j, :] for j in range(wtiles))
                        for j in range(wtiles):
                            k = k0 + j
                            t = pa.tile([128, L], BF16, tag=f"xa{k}",
                                        name=f"xa{k}")
                            nc.sync.dma_start(
                                t[:], xT_d[k * 128:(k + 1) * 128, 0:L])
                            xa.append(t)
                        k0 += wtiles
                    for kc in range(KD // 4):
                        t = pa.tile([128, 4, L], BF16, tag=f"xb{kc}",
                                    name=f"xb{kc}")
                        nc.sync.dma_start(
                            t[:], xT_d[kc * 512:(kc + 1) * 512, L:S_LOC]
                            .rearrange("(j p) c -> p j c", p=128))
                        xb.extend(t[:, j, :] for j in range(4))
                    xT = [xa, xb]   # xT[b][k]: [128, L] block-b half
                    # one merged DMA for all of wk (16x fewer HWDGE holds);
                    # K proj starts ~27us in, so the single semaphore is fine
                    wk_all = pkv.tile([128, KD, HKV * DH], BF16, tag="wk_all",
                                      name="wk_all")
                    nc.sync.dma_start(
                        wk_all[:],
                        wkT_d.rearrange("(k p) c -> p k c", p=128))
                    wkT = [wk_all[:, k, :] for k in range(KD)]
                    nc.sync.dma_start(c1[:], c1_d[:])
                    nc.sync.dma_start(c2[:], c2_d[:])
                    # slab0 borrows the wo0 slot of the wo-prefetch pool:
                    # fresh SBUF, so its DMA is not blocked by the pool-reuse
                    # barrier that delays the wp-ring slabs until K-proj ends
                    slab0 = pwo.tile([128, KD, 512], BF16, tag="wo0",
                                     name="slab0")
                    nc.sync.dma_start(
                        slab0[:], wqT_d.rearrange("(k p) c -> p k c", p=128)
                        [:, :, 0:512])

                    # ---- V projection ----
                    # k-major in two 4-bank passes: each arriving (xT, wvT)
                    # DMA pair enables 4 matmuls immediately, limiting the
                    # PE's DMA-paced serialization at kernel start
                    for half in range(2):
                        vms = [psA.tile([128, HKV * DH], F32, tag="aps",
                                        name="aps") for _ in range(4)]
                        # NOTE: do NOT split these into narrower column
                        # chunks with interleaved accumulation groups —
                        # concurrent open groups within one PSUM bank
                        # corrupt results on real HW (sim won't catch it).
                        for k in range(KD):
                            for j, vm in enumerate(vms):
                                m = half * 4 + j
                                nc.tensor.matmul(
                                    vm[:],
                                    xT[half][k][:, j * 128:(j + 1) * 128],
                                    wvT[k][:],
                                    start=(k == 0), stop=(k == KD - 1))
                        for j, vm in enumerate(vms):
                            m = half * 4 + j
                            nc.vector.memset(vaug[m][:], 1.0)
                            nc.vector.tensor_copy(
                                vaug[m].rearrange("p (g d) -> p g d",
                                                  d=DH + 1)[:, :, 0:DH],
                                vm.rearrange("p (g d) -> p g d", d=DH))

                    # ---- K projection + rope ----
                    for i in range(NKI):
                        for b in range(NBLK):
                            ps = psA.tile([128, L], F32, tag="aps", name="aps")
                            for k in range(KD):
                                nc.tensor.matmul(
                                    ps[:], wkT[k][:, i * 128:(i + 1) * 128],
                                    xT[b][k][:],
                                    start=(k == 0), stop=(k == KD - 1))
                            rope_epilogue(ps, b, kt[i][b])
                # pkv closes here: wk/wv SBUF is recycled for the wo prefetch

                with (
                    tc.tile_pool(name="wslab", bufs=1) as wp,
                    tc.tile_pool(name="probs", bufs=5) as probp,
                    tc.tile_pool(name="psS", bufs=3, space="PSUM") as psS,
                    tc.tile_pool(name="psO", bufs=2, space="PSUM") as psO,
                ):
                    _build_phase_b(nc, tc, sp, wp, probp, psA, psS, psO, pwo,
                                   xT, c1, c2, kt, vaug, att, woT, slab0,
                                   wqT_d, woT_d, out_d, rope_epilogue,
                                   KD, NQI, NMT, EXP)

            # ================= phase C: output projection =================
            with (
                tc.tile_pool(name="outsb", bufs=4) as op,
                tc.tile_pool(name="psC", bufs=4, space="PSUM") as psC,
            ):
                for b in range(NBLK):
                    for st in range(NMT):           # token tile within block
                        if b == 0 and st == 0:
                            continue        # emitted early, end of phase B
                        rows = slice(b * L + st * 128, b * L + (st + 1) * 128)
                        last = (b == NBLK - 1 and st == NMT - 1)
                        if not last:
                            # merged store: 4 psum tiles staged into one
                            # [128, 2048] buffer, single DMA (1 HWDGE hold)
                            ot = op.tile([128, DIM], F32, tag="ot", name="ot",
                                         bufs=2)
                            for ec in range(DIM // 512):
                                ps = psC.tile([128, 512], F32, tag="ops",
                                              name="ops_c")
                                for k in range(NQI):
                                    nc.tensor.matmul(
                                        ps[:],
                                        att[k][b][:, st * 128:(st + 1) * 128],
                                        woT[k][:, ec * 512:(ec + 1) * 512],
                                        start=(k == 0), stop=(k == NQI - 1))
                                nc.vector.tensor_copy(
                                    ot[:, ec * 512:(ec + 1) * 512], ps[:])
                            nc.sync.dma_start(out_d[rows, :], ot[:])
                        else:
                            # drain tail: narrow tiles, store-per-tile so the
                            # final copy+DMA after the last matmul is short
                            for ec in range(DIM // 256):
                                ps = psC.tile([128, 256], F32, tag="ops_t",
                                              name="ops_c")
                                for k in range(NQI):
                                    nc.tensor.matmul(
                                        ps[:],
                                        att[k][b][:, st * 128:(st + 1) * 128],
                                        woT[k][:, ec * 256:(ec + 1) * 256],
                                        start=(k == 0), stop=(k == NQI - 1))
                                ott = op.tile([128, 256], F32, tag="ot_t",
                                              name="ott", bufs=4)
                                nc.vector.tensor_copy(ott[:], ps[:])
                                nc.sync.dma_start(out_d[rows, ec * 256:(ec + 1) * 256],
                                                  ott[:])

    nc.compile()
    return nc


def _build_phase_b(nc, tc, sp, wp, probp, psA, psS, psO, pwo,
                   xT, c1, c2, kt, vaug, att, woT, slab0,
                   wqT_d, woT_d, out_d, rope_epilogue, KD, NQI, NMT, EXP):
    """Q projection + attention, interleaved; prefetches wq slabs one head
    group ahead and wo tiles across the phase."""

    def attention(qtile, hp, b):
        """scores/softmax/PV-transposed/normalize for head pair hp, block b.

        PV is computed transposed (oT[q, d] with q on partitions) so every
        PV matmul has full M=128 output partitions and only N=65 moving
        columns: half the PE rows of the [65, L] orientation. The softmax
        denominator (ones-column of vaug) then lands as a per-partition
        column, so normalization is a strided reciprocal + one broadcast
        multiply per head (no gpsimd partition_broadcast). The
        [q, d] -> [d, q] layout fix for the output projection is done by
        the XBAR DMA transpose (DMA engines, not PE).
        """
        gi = hp // 4
        probs_e, probs_o = [], []
        for mi in range(NMT):
            se = psS.tile([128, L], F32, tag="s", name="sps")
            nc.tensor.matmul(
                se[:],
                kt[gi][b][0:64, mi * 128:(mi + 1) * 128],
                qtile[0:64, :])
            so = psS.tile([128, L], F32, tag="s", name="sps")
            nc.tensor.matmul(
                so[:],
                kt[gi][b][64:128, mi * 128:(mi + 1) * 128],
                qtile[64:128, :])
            pe = probp.tile([128, L], BF16, tag="pe", name="pe")
            nc.scalar.activation(pe[:], se[:], EXP, scale=SCALE)
            po = probp.tile([128, L], BF16, tag="po", name="po")
            nc.scalar.activation(po[:], so[:], EXP, scale=SCALE)
            probs_e.append(pe)
            probs_o.append(po)
        ge, go = 2 * gi, 2 * gi + 1
        qd = sp.tile([128, NMT, 128], BF16, tag="qd", name="qd", bufs=3)
        for h, (probs, g) in enumerate(((probs_e, ge), (probs_o, go))):
            oT = psO.tile([128, NMT * (DH + 1)], F32, tag="o", name="ops_o")
            for qs in range(NMT):
                for mi in range(NMT):
                    nc.tensor.matmul(
                        oT[:, qs * (DH + 1):(qs + 1) * (DH + 1)],
                        probs[mi][:, qs * 128:(qs + 1) * 128],
                        vaug[b * NMT + mi][:, g * (DH + 1):(g + 1) * (DH + 1)],
                        start=(mi == 0), stop=(mi == NMT - 1))
            o3 = oT.rearrange("p (qs c) -> p qs c", c=DH + 1)
            rc = sp.tile([128, NMT, 1], F32, tag="rc", name="rc")
            nc.vector.reciprocal(rc[:], o3[:, :, DH:DH + 1])
            in0, in1 = bass.broadcast_tensor_aps(o3[:, :, 0:DH], rc[:])
            nc.vector.tensor_mul(qd[:, :, h * DH:(h + 1) * DH], in0, in1)
        for qs in range(NMT):
            nc.sync.dma_start(
                att[hp][b][:, qs * 128:(qs + 1) * 128],
                qd[:, qs:qs + 1, :], transpose=True)

    def load_slab(ig):
        """Double-buffered wq slab load as ONE merged DMA (single HWDGE
        hold), issued one group ahead so it runs before that group's xbar
        transposes in SP-queue order (no Q-proj starvation at group
        entry)."""
        t = wp.tile([128, KD, 512], BF16, tag="wq", name="wq", bufs=2)
        nc.sync.dma_start(
            t[:], wqT_d.rearrange("(k p) c -> p k c", p=128)
            [:, :, ig * 512:(ig + 1) * 512])
        return [t[:, k, :] for k in range(KD)]

    def load_wo(ig):
        """Prefetch 4 wo row-tiles per head group (one merged DMA) into the
        SBUF freed by the k/v weights, so phase C starts with wo resident."""
        t = pwo.tile([128, 4, DIM], BF16, tag=f"wo{ig}", name=f"wo{ig}")
        nc.sync.dma_start(
            t[:], woT_d[ig * 512:(ig + 1) * 512, :]
            .rearrange("(j p) c -> p j c", p=128))
        for j in range(4):
            woT[4 * ig + j] = t[:, j, :]

    def early_chain(st, ec):
        """One (b=0, st, ec) output-projection chain, emitted at the end of
        phase B to fill the PE idle left by the final attentions' exp/PV/
        transpose latency (no further Q-proj exists to hide it). Phase C
        skips these."""
        ps = psA.tile([128, L], F32, tag="aps", name="aps")
        for k in range(NQI):
            nc.tensor.matmul(
                ps[:], att[k][0][:, st * 128:(st + 1) * 128],
                woT[k][:, ec * 512:(ec + 1) * 512],
                start=(k == 0), stop=(k == NQI - 1))
        ot = sp.tile([128, 512], F32, tag="ot_e", name="ot_e")
        nc.vector.tensor_copy(ot[:], ps[:])
        nc.sync.dma_start(
            out_d[st * 128:(st + 1) * 128, ec * 512:(ec + 1) * 512], ot[:])

    # slab0 was loaded in phase A (borrowing the wo0 slot); wo prefetches are
    # shifted one group late so the wo0 DMA's slot-reuse wait (on ig0's
    # Q-proj reads of slab0) is already satisfied when it's issued.
    slab_next = [slab0[:, k, :] for k in range(KD)]
    for ig in range(4):
        slab = slab_next
        if ig + 1 < 4:
            slab_next = load_slab(ig + 1)
        if ig >= 1:
            load_wo(ig - 1)
        for ii in range(4):
            i = ig * 4 + ii
            for b in range(NBLK):
                ps = psA.tile([128, L], F32, tag="aps", name="aps")
                for k in range(KD):
                    nc.tensor.matmul(
                        ps[:], slab[k][:, ii * 128:(ii + 1) * 128],
                        xT[b][k][:],
                        start=(k == 0), stop=(k == KD - 1))
                qtile = sp.tile([128, L], BF16, tag="qt", name="qt", bufs=2)
                rope_epilogue(ps, b, qtile)
                attention(qtile, i, b)
                if ig == 3 and ii == 3 and b == 0:
                    early_chain(0, 0)
                    early_chain(0, 1)
            if ig == 3 and ii == 1:
                load_wo(3)    # last wo group: fresh slot, loads during ig3
    early_chain(0, 2)
    early_chain(0, 3)


def _prep_shared(wq, wk, wv, wo):
    bf = ml_dtypes.bfloat16

    # wq: head order HPERM, PERM64 within head
    wq_p = wq.reshape(H, DH, DIM)[HPERM][:, PERM64, :].reshape(H * DH, DIM)
    # wk: natural head order, PERM64 within head
    wk_p = wk.reshape(HKV, DH, DIM)[:, PERM64, :].reshape(HKV * DH, DIM)
    # wo columns: head order HPERM, dims unpermuted (V is not roped)
    wo_p = wo.reshape(DIM, H, DH)[:, HPERM, :].reshape(DIM, H * DH)

    wqT = np.ascontiguousarray(wq_p.T.astype(bf))
    wkT = np.ascontiguousarray(wk_p.T.astype(bf))
    wvT = np.ascontiguousarray(wv.T.astype(bf))
    woT = np.ascontiguousarray(wo_p.T.astype(bf))
    return wqT, wkT, wvT, woT


def kernel(x, freqs_cos, freqs_sin, wq, wk, wv, wo):
    global LAST_RESULTS
    x = np.asarray(x, np.float32)
    freqs_cos = np.asarray(freqs_cos, np.float32)
    freqs_sin = np.asarray(freqs_sin, np.float32)
    bf = ml_dtypes.bfloat16

    if "nc" not in _CACHED:
        _CACHED["nc"] = _build()
    nc = _CACHED["nc"]

    wqT, wkT, wvT, woT = _prep_shared(
        np.asarray(wq, np.float32), np.asarray(wk, np.float32),
        np.asarray(wv, np.float32), np.asarray(wo, np.float32))

    in_maps = []
    for c in range(NCORE):
        rows = slice(c * S_LOC, (c + 1) * S_LOC)
        xT = np.ascontiguousarray(x[rows].T.astype(bf))
        fcc = freqs_cos[rows]      # [S_LOC, 32]
        fss = freqs_sin[rows]
        c1h = fcc[:, FREQ_IDX].T   # [64, S_LOC]
        c2h = (fss[:, FREQ_IDX] * C2_SIGN[None, :]).T
        c1 = np.ascontiguousarray(np.concatenate([c1h, c1h], 0).astype(bf))
        c2 = np.ascontiguousarray(np.concatenate([c2h, c2h], 0).astype(bf))
        in_maps.append({"xT": xT, "wqT": wqT, "wkT": wkT, "wvT": wvT,
                        "woT": woT, "c1": c1, "c2": c2})

    res = None
    for attempt in range(3):
        try:
            res = run_bass_kernel_spmd(nc, in_maps, list(range(NCORE)))
            break
        except Exception:
            if attempt == 2:
                raise
            import time
            time.sleep(10)   # transient NRT device errors usually clear on retry
    LAST_RESULTS = res
    out = np.concatenate([res.results[c]["out"] for c in range(NCORE)], axis=0)
    return np.ascontiguousarray(out.astype(np.float32))



# revision 54
# speedup vs baseline: 1.0139x; 1.0139x over previous
"""Trainium2 Bass kernel for packed-sequence GQA attention (nn_Attention_84602265796942).

Sharding: data-parallel over the B=16 packed sequences -> 2 sequences (1024
tokens) per core, weights replicated. Zero collectives.

Per-core pipeline (all matmuls bf16 x bf16 -> fp32 PSUM):
  A) QKV projections from host-transposed inputs; RoPE applied via a
     host-side head-dim permutation ([a0..a15 b0..b15 a16..a31 b16..b31])
     so the rotation partner sits at partition r^16 (one stream_shuffle).
     x/wv loads are interleaved (ramped merged wv chunks) and V runs
     k-major in 4-bank passes so the PE tracks DMA arrival; wk is one
     merged DMA.
  B) Attention per (block, head-pair), emitted interleaved with the Q
     projection so PE/ACT/DVE overlap: q-heads are host-permuted so pair
     (h, h+4) shares a qt tile and maps to kv heads (2g, 2g+1) = the two
     partition halves of one K tile; scores are computed transposed
     (scoresT[m,l]) with paired K=64 matmuls at partition bases 0/64;
     softmax without max subtraction (scores are bounded); PV is computed
     TRANSPOSED (oT[q, 65] = probsT @ V_aug, full M=128 output
     partitions, N=65 columns -- half the PE rows of the [65, L]
     orientation); the ones-column row-sums land per-partition, so
     normalization is a strided reciprocal + broadcast multiply; the
     [q, d] -> [d, q] fix-up for the output projection runs on the XBAR
     DMA transpose (DMA engines, not PE). wq slabs are double-buffered
     and prefetched a head-group ahead (slab0 borrows the wo0 slot so it
     loads during phase A); wo is prefetched across phase B into the
     SBUF freed by wk/wv.
  C) Output projection; the first token tile's chains are emitted at the
     end of phase B to fill the final attentions' latency; merged
     [128, 2048] stores except a narrow-tile drain for the last token
     tile.

PSUM pools are 3 (proj) + 3 (scores) + 2 (PV) banks in phases A/B and
4+4 in phase C. Accumulation-group discipline: one open group at a time
per bank region (concurrent interleaved groups in a bank corrupt results
on real HW even though the timeline simulator accepts them).

A dozen dependency-free warm-up matmuls (zeros, discarded) precede the
V projection: they hold the PE's p-state ramp through the initial DMA
wait so the first real matmuls run at full clock. The remaining startup
window is input-delivery-bound (x+wv+wk bytes at DMA bus rate), and the
phase-B steady state is gated by ACT exp throughput (8 x 612ns per
head-pair vs a 6.1us PE iteration) -- both at their structural floors.
"""
import numpy as np
import ml_dtypes

import concourse.bass as bass
import concourse.tile as tile
from concourse import bacc, mybir
from concourse.bass_utils import run_bass_kernel_spmd

F32 = mybir.dt.float32
BF16 = mybir.dt.bfloat16

B, L, DIM, H, HKV, DH = 16, 512, 2048, 32, 8, 64
REP = H // HKV
S = B * L
NCORE = 8
S_LOC = S // NCORE          # 1024 tokens per core
NBLK = S_LOC // L           # 2 blocks per core
SCALE = DH ** -0.5

# within-head dim permutation: rows [a0..a15, b0..b15, a16..a31, b16..b31]
PERM64 = np.concatenate([np.arange(0, 32, 2), np.arange(1, 32, 2),
                         np.arange(32, 64, 2), np.arange(33, 64, 2)])
_rr = np.arange(64)
FREQ_IDX = (_rr // 32) * 16 + (_rr % 16)
C2_SIGN = np.where((_rr % 32) < 16, -1.0, 1.0).astype(np.float32)
# q-head order: pair (h, h+4) within each group of 8 -> kv heads (2g, 2g+1)
HPERM = np.array([8 * gi + t + 4 * half
                  for gi in range(4) for t in range(4) for half in range(2)])

_CACHED = {}

LAST_RESULTS = None  # BassKernelResults of the most recent run (for test.py)


def _build():
    nc = bacc.Bacc("TRN2", target_bir_lowering=False, debug=False,
                   num_devices=NCORE)

    xT_d = nc.dram_tensor("xT", [DIM, S_LOC], BF16, kind="ExternalInput")
    wqT_d = nc.dram_tensor("wqT", [DIM, H * DH], BF16, kind="ExternalInput")
    wkT_d = nc.dram_tensor("wkT", [DIM, HKV * DH], BF16, kind="ExternalInput")
    wvT_d = nc.dram_tensor("wvT", [DIM, HKV * DH], BF16, kind="ExternalInput")
    woT_d = nc.dram_tensor("woT", [H * DH, DIM], BF16, kind="ExternalInput")
    c1_d = nc.dram_tensor("c1", [128, S_LOC], BF16, kind="ExternalInput")
    c2_d = nc.dram_tensor("c2", [128, S_LOC], BF16, kind="ExternalInput")
    out_d = nc.dram_tensor("out", [S_LOC, DIM], F32, kind="ExternalOutput")

    KD = DIM // 128          # 16 contraction tiles
    NQI = (H * DH) // 128    # 16 Q row-tiles (one head pair each)
    NKI = (HKV * DH) // 128  # 4 K row-tiles
    NMT = L // 128           # 4 token tiles per block
    EXP = mybir.ActivationFunctionType.Exp
    NWARM = 12
    XACHUNK = (2,) * 8
    SHUF_MASK = [i ^ 16 for i in range(32)]

    with tile.TileContext(nc) as tc:
        with (
            tc.tile_pool(name="persist", bufs=1) as pp,      # long-lived activations
            tc.tile_pool(name="scratch", bufs=2) as sp,      # rope/norm scratch
            tc.tile_pool(name="wop", bufs=1) as pwo,         # wo, prefetched in phase B
        ):
            # persistent activation tensors (qt is a short-lived ring in sp)
            kt = [[pp.tile([128, L], BF16, tag=f"kt{g}_{b}", name=f"kt{g}_{b}")
                   for b in range(NBLK)] for g in range(NKI)]
            vaug = [pp.tile([128, HKV * (DH + 1)], BF16, tag=f"va{m}", name=f"va{m}")
                    for m in range(S_LOC // 128)]
            att = [[pp.tile([128, L], BF16, tag=f"at{i}_{b}", name=f"at{i}_{b}")
                    for b in range(NBLK)] for i in range(NQI)]
            woT = [None] * NQI

            # ================= phase A: QKV projections + RoPE =================
            with (
                tc.tile_pool(name="inA", bufs=1) as pa,
                tc.tile_pool(name="psA", bufs=3, space="PSUM") as psA,
            ):
                def rope_epilogue(ps, b, dst128):
                    """ps: [128, 512] psum of pre-rope QT/KT rows -> bf16 dst.

                    4 DVE ops: t1 and the shuffle both read PSUM directly
                    (no staging copy; the psA slot frees after the second
                    reader, well within the 4-slot rotation slack)."""
                    cs = slice(b * L, (b + 1) * L)
                    t1 = sp.tile([128, L], BF16, tag="t1", name="t1")
                    nc.vector.tensor_mul(t1[:], ps[:], c1[:, cs])
                    sh = sp.tile([128, L], F32, tag="sh", name="sh")
                    nc.vector.stream_shuffle(sh[:], ps[:], SHUF_MASK)
                    t2 = sp.tile([128, L], BF16, tag="t2", name="t2")
                    nc.vector.tensor_mul(t2[:], sh[:], c2[:, cs])
                    nc.vector.tensor_add(dst128[:], t1[:], t2[:])

                xT = []
                c1 = pa.tile([128, S_LOC], BF16, tag="c1", name="c1s")
                c2 = pa.tile([128, S_LOC], BF16, tag="c2", name="c2s")

                with tc.tile_pool(name="inKV", bufs=1) as pkv:
                    wkT, wvT = [], []
                    # x is loaded in block halves: the block-0 half tiles
                    # (xa) interleave with merged wv chunks so V-proj pass 1
                    # (m 0..3, block-0 tokens) is PE-bound from the first
                    # pair; the block-1 halves (xb) follow as merged chunks
                    xa, xb = [], []
                    k0 = 0
                    for nw, wtiles in enumerate(XACHUNK):
                        # ramped merged chunks for BOTH wv and xa: the xa
                        # stream as single tiles is HWDGE-hold-bound (625ns
                        # per DMA > the 364ns transfer), which paces V pass 1
                        wvc = pkv.tile([128, wtiles, HKV * DH], BF16,
                                       tag=f"wvT{nw}", name=f"wvT{nw}")
                        nc.sync.dma_start(
                            wvc[:],
                            wvT_d[k0 * 128:(k0 + wtiles) * 128, :]
                            .rearrange("(j p) c -> p j c", p=128))
                        wvT.extend(wvc[:, j, :] for j in range(wtiles))
                        xac = pa.tile([128, wtiles, L], BF16, tag=f"xa{nw}",
                                      name=f"xa{nw}")
                        nc.sync.dma_start(
                            xac[:], xT_d[k0 * 128:(k0 + wtiles) * 128, 0:L]
                            .rearrange("(j p) c -> p j c", p=128))
                        xa.extend(xac[:, j, :] for j in range(wtiles))
                        k0 += wtiles
                    XBW = 8
                    for kc in range(KD // XBW):
                        t = pa.tile([128, XBW, L], BF16, tag=f"xb{kc}",
                                    name=f"xb{kc}")
                        nc.sync.dma_start(
                            t[:],
                            xT_d[kc * 128 * XBW:(kc + 1) * 128 * XBW, L:S_LOC]
                            .rearrange("(j p) c -> p j c", p=128))
                        xb.extend(t[:, j, :] for j in range(XBW))
                    xT = [xa, xb]   # xT[b][k]: [128, L] block-b half
                    # one merged DMA for all of wk (16x fewer HWDGE holds);
                    # K proj starts ~27us in, so the single semaphore is fine
                    wk_all = pkv.tile([128, KD, HKV * DH], BF16, tag="wk_all",
                                      name="wk_all")
                    nc.sync.dma_start(
                        wk_all[:],
                        wkT_d.rearrange("(k p) c -> p k c", p=128))
                    wkT = [wk_all[:, k, :] for k in range(KD)]
                    nc.sync.dma_start(c1[:], c1_d[:])
                    nc.sync.dma_start(c2[:], c2_d[:])
                    # slab0 borrows the wo0 slot of the wo-prefetch pool:
                    # fresh SBUF, so its DMA is not blocked by the pool-reuse
                    # barrier that delays the wp-ring slabs until K-proj ends
                    slab0 = pwo.tile([128, KD, 512], BF16, tag="wo0",
                                     name="slab0")
                    nc.sync.dma_start(
                        slab0[:], wqT_d.rearrange("(k p) c -> p k c", p=128)
                        [:, :, 0:512])

                    # ---- PE p-state warm-up ----
                    # dependency-free throwaway matmuls (zeros from a memset
                    # tile, discarded psum region) keep the PE's busy-run
                    # alive through the initial DMA wait, so the real V
                    # matmuls are dispatched with ramp time > 3us and run at
                    # full clock instead of the LOW/MID p-states
                    wsrc = sp.tile([128, 128], BF16, tag="warm", name="wsrc",
                                   bufs=1)
                    nc.vector.memset(wsrc[:], 0.0)
                    wps = psA.tile([128, L], F32, tag="aps", name="wps")
                    for _ in range(NWARM):
                        nc.tensor.matmul(wps[:, 0:128], wsrc[:], wsrc[:])

                    # ---- V projection ----
                    # k-major in two 4-bank passes: each arriving (xT, wvT)
                    # DMA pair enables 4 matmuls immediately, limiting the
                    # PE's DMA-paced serialization at kernel start
                    for half in range(2):
                        vms = [psA.tile([128, HKV * DH], F32, tag="aps",
                                        name="aps") for _ in range(4)]
                        # NOTE: do NOT split these into narrower column
                        # chunks with interleaved accumulation groups —
                        # concurrent open groups within one PSUM bank
                        # corrupt results on real HW (sim won't catch it).
                        for k in range(KD):
                            for j, vm in enumerate(vms):
                                m = half * 4 + j
                                nc.tensor.matmul(
                                    vm[:],
                                    xT[half][k][:, j * 128:(j + 1) * 128],
                                    wvT[k][:],
                                    start=(k == 0), stop=(k == KD - 1))
                        for j, vm in enumerate(vms):
                            m = half * 4 + j
                            nc.vector.memset(vaug[m][:], 1.0)
                            nc.vector.tensor_copy(
                                vaug[m].rearrange("p (g d) -> p g d",
                                                  d=DH + 1)[:, :, 0:DH],
                                vm.rearrange("p (g d) -> p g d", d=DH))

                    # ---- K projection + rope ----
                    for i in range(NKI):
                        for b in range(NBLK):
                            ps = psA.tile([128, L], F32, tag="aps", name="aps")
                            for k in range(KD):
                                nc.tensor.matmul(
                                    ps[:], wkT[k][:, i * 128:(i + 1) * 128],
                                    xT[b][k][:],
                                    start=(k == 0), stop=(k == KD - 1))
                            rope_epilogue(ps, b, kt[i][b])
                # pkv closes here: wk/wv SBUF is recycled for the wo prefetch

                with (
                    tc.tile_pool(name="wslab", bufs=1) as wp,
                    tc.tile_pool(name="probs", bufs=5) as probp,
                    tc.tile_pool(name="psS", bufs=3, space="PSUM") as psS,
                    tc.tile_pool(name="psO", bufs=2, space="PSUM") as psO,
                ):
                    _build_phase_b(nc, tc, sp, wp, probp, psA, psS, psO, pwo,
                                   xT, c1, c2, kt, vaug, att, woT, slab0,
                                   wqT_d, woT_d, out_d, rope_epilogue,
                                   KD, NQI, NMT, EXP)

            # ================= phase C: output projection =================
            with (
                tc.tile_pool(name="outsb", bufs=4) as op,
                tc.tile_pool(name="psC", bufs=4, space="PSUM") as psC,
            ):
                for b in range(NBLK):
                    for st in range(NMT):           # token tile within block
                        if b == 0 and st == 0:
                            continue        # emitted early, end of phase B
                        rows = slice(b * L + st * 128, b * L + (st + 1) * 128)
                        last = (b == NBLK - 1 and st == NMT - 1)
                        if not last:
                            # merged store: 4 psum tiles staged into one
                            # [128, 2048] buffer, single DMA (1 HWDGE hold)
                            ot = op.tile([128, DIM], F32, tag="ot", name="ot",
                                         bufs=2)
                            for ec in range(DIM // 512):
                                ps = psC.tile([128, 512], F32, tag="ops",
                                              name="ops_c")
                                for k in range(NQI):
                                    nc.tensor.matmul(
                                        ps[:],
                                        att[k][b][:, st * 128:(st + 1) * 128],
                                        woT[k][:, ec * 512:(ec + 1) * 512],
                                        start=(k == 0), stop=(k == NQI - 1))
                                nc.vector.tensor_copy(
                                    ot[:, ec * 512:(ec + 1) * 512], ps[:])
                            nc.sync.dma_start(out_d[rows, :], ot[:])
                        else:
                            # drain tail: narrow tiles, store-per-tile so the
                            # final copy+DMA after the last matmul is short
                            for ec in range(DIM // 256):
                                ps = psC.tile([128, 256], F32, tag="ops_t",
                                              name="ops_c")
                                for k in range(NQI):
                                    nc.tensor.matmul(
                                        ps[:],
                                        att[k][b][:, st * 128:(st + 1) * 128],
                                        woT[k][:, ec * 256:(ec + 1) * 256],
                                        start=(k == 0), stop=(k == NQI - 1))
                                ott = op.tile([128, 256], F32, tag="ot_t",
                                              name="ott", bufs=4)
                                nc.vector.tensor_copy(ott[:], ps[:])
                                nc.sync.dma_start(out_d[rows, ec * 256:(ec + 1) * 256],
                                                  ott[:])

    nc.compile()
    return nc


def _build_phase_b(nc, tc, sp, wp, probp, psA, psS, psO, pwo,
                   xT, c1, c2, kt, vaug, att, woT, slab0,
                   wqT_d, woT_d, out_d, rope_epilogue, KD, NQI, NMT, EXP):
    """Q projection + attention, interleaved; prefetches wq slabs one head
    group ahead and wo tiles across the phase."""

    def attention(qtile, hp, b):
        """scores/softmax/PV-transposed/normalize for head pair hp, block b.

        PV is computed transposed (oT[q, d] with q on partitions) so every
        PV matmul has full M=128 output partitions and only N=65 moving
        columns: half the PE rows of the [65, L] orientation. The softmax
        denominator (ones-column of vaug) then lands as a per-partition
        column, so normalization is a strided reciprocal + one broadcast
        multiply per head (no gpsimd partition_broadcast). The
        [q, d] -> [d, q] layout fix for the output projection is done by
        the XBAR DMA transpose (DMA engines, not PE).
        """
        gi = hp // 4
        qd = sp.tile([128, NMT, 128], BF16, tag="qd", name="qd", bufs=3)
        for half, tag in ((0, "pe"), (1, "po")):
            # per head: scores+exps then immediately its PV accumulation,
            # so the PE runs head-e PV while ACT drains head-o's exps
            probs = []
            for mi in range(NMT):
                sx = psS.tile([128, L], F32, tag="s", name="sps")
                nc.tensor.matmul(
                    sx[:],
                    kt[gi][b][half * 64:(half + 1) * 64,
                              mi * 128:(mi + 1) * 128],
                    qtile[half * 64:(half + 1) * 64, :])
                px = probp.tile([128, L], BF16, tag=tag, name="px")
                nc.scalar.activation(px[:], sx[:], EXP, scale=SCALE)
                probs.append(px)
            g = 2 * gi + half
            oT = psO.tile([128, NMT * (DH + 1)], F32, tag="o", name="ops_o")
            for qs in range(NMT):
                for mi in range(NMT):
                    nc.tensor.matmul(
                        oT[:, qs * (DH + 1):(qs + 1) * (DH + 1)],
                        probs[mi][:, qs * 128:(qs + 1) * 128],
                        vaug[b * NMT + mi][:, g * (DH + 1):(g + 1) * (DH + 1)],
                        start=(mi == 0), stop=(mi == NMT - 1))
            o3 = oT.rearrange("p (qs c) -> p qs c", c=DH + 1)
            rc = sp.tile([128, NMT, 1], F32, tag="rc", name="rc")
            nc.vector.reciprocal(rc[:], o3[:, :, DH:DH + 1])
            in0, in1 = bass.broadcast_tensor_aps(o3[:, :, 0:DH], rc[:])
            nc.vector.tensor_mul(qd[:, :, half * DH:(half + 1) * DH],
                                 in0, in1)
        for qs in range(NMT):
            nc.sync.dma_start(
                att[hp][b][:, qs * 128:(qs + 1) * 128],
                qd[:, qs:qs + 1, :], transpose=True)

    def load_slab(ig):
        """Double-buffered wq slab load as ONE merged DMA (single HWDGE
        hold), issued one group ahead so it runs before that group's xbar
        transposes in SP-queue order (no Q-proj starvation at group
        entry)."""
        t = wp.tile([128, KD, 512], BF16, tag="wq", name="wq", bufs=2)
        nc.sync.dma_start(
            t[:], wqT_d.rearrange("(k p) c -> p k c", p=128)
            [:, :, ig * 512:(ig + 1) * 512])
        return [t[:, k, :] for k in range(KD)]

    def load_wo(ig):
        """Prefetch 4 wo row-tiles per head group (one merged DMA) into the
        SBUF freed by the k/v weights, so phase C starts with wo resident."""
        t = pwo.tile([128, 4, DIM], BF16, tag=f"wo{ig}", name=f"wo{ig}")
        nc.sync.dma_start(
            t[:], woT_d[ig * 512:(ig + 1) * 512, :]
            .rearrange("(j p) c -> p j c", p=128))
        for j in range(4):
            woT[4 * ig + j] = t[:, j, :]

    def early_chain(st, ec, pool=None, tag="aps"):
        """One (b=0, st, ec) output-projection chain, emitted at the end of
        phase B to fill the PE idle left by the final attentions' exp/PV/
        transpose latency (no further Q-proj exists to hide it). Phase C
        skips these."""
        ps = (pool or psA).tile([128, L], F32, tag=tag, name="eps")
        for k in range(NQI):
            nc.tensor.matmul(
                ps[:], att[k][0][:, st * 128:(st + 1) * 128],
                woT[k][:, ec * 512:(ec + 1) * 512],
                start=(k == 0), stop=(k == NQI - 1))
        ot = sp.tile([128, 512], F32, tag="ot_e", name="ot_e")
        nc.vector.tensor_copy(ot[:], ps[:])
        nc.sync.dma_start(
            out_d[st * 128:(st + 1) * 128, ec * 512:(ec + 1) * 512], ot[:])

    # slab0 was loaded in phase A (borrowing the wo0 slot); wo prefetches are
    # shifted one group late so the wo0 DMA's slot-reuse wait (on ig0's
    # Q-proj reads of slab0) is already satisfied when it's issued.
    slab_next = [slab0[:, k, :] for k in range(KD)]
    for ig in range(4):
        slab = slab_next
        if ig + 1 < 4:
            slab_next = load_slab(ig + 1)
        if ig >= 1:
            load_wo(ig - 1)
        for ii in range(4):
            i = ig * 4 + ii
            for b in range(NBLK):
                ps = psA.tile([128, L], F32, tag="aps", name="aps")
                for k in range(KD):
                    nc.tensor.matmul(
                        ps[:], slab[k][:, ii * 128:(ii + 1) * 128],
                        xT[b][k][:],
                        start=(k == 0), stop=(k == KD - 1))
                qtile = sp.tile([128, L], BF16, tag="qt", name="qt", bufs=2)
                rope_epilogue(ps, b, qtile)
                attention(qtile, i, b)
                if ig == 3 and ii == 3 and b == 0:
                    early_chain(0, 0)
            if ig == 3 and ii == 1:
                load_wo(3)    # last wo group: fresh slot, loads during ig3
    early_chain(0, 1)
    early_chain(0, 2)
    early_chain(0, 3)


def _prep_shared(wq, wk, wv, wo):
    bf = ml_dtypes.bfloat16

    # wq: head order HPERM, PERM64 within head
    wq_p = wq.reshape(H, DH, DIM)[HPERM][:, PERM64, :].reshape(H * DH, DIM)
    # wk: natural head order, PERM64 within head
    wk_p = wk.reshape(HKV, DH, DIM)[:, PERM64, :].reshape(HKV * DH, DIM)
    # wo columns: head order HPERM, dims unpermuted (V is not roped)
    wo_p = wo.reshape(DIM, H, DH)[:, HPERM, :].reshape(DIM, H * DH)

    wqT = np.ascontiguousarray(wq_p.T.astype(bf))
    wkT = np.ascontiguousarray(wk_p.T.astype(bf))
    wvT = np.ascontiguousarray(wv.T.astype(bf))
    woT = np.ascontiguousarray(wo_p.T.astype(bf))
    return wqT, wkT, wvT, woT


def kernel(x, freqs_cos, freqs_sin, wq, wk, wv, wo):
    global LAST_RESULTS
    x = np.asarray(x, np.float32)
    freqs_cos = np.asarray(freqs_cos, np.float32)
    freqs_sin = np.asarray(freqs_sin, np.float32)
    bf = ml_dtypes.bfloat16

    if "nc" not in _CACHED:
        _CACHED["nc"] = _build()
    nc = _CACHED["nc"]

    wqT, wkT, wvT, woT = _prep_shared(
        np.asarray(wq, np.float32), np.asarray(wk, np.float32),
        np.asarray(wv, np.float32), np.asarray(wo, np.float32))

    in_maps = []
    for c in range(NCORE):
        rows = slice(c * S_LOC, (c + 1) * S_LOC)
        xT = np.ascontiguousarray(x[rows].T.astype(bf))
        fcc = freqs_cos[rows]      # [S_LOC, 32]
        fss = freqs_sin[rows]
        c1h = fcc[:, FREQ_IDX].T   # [64, S_LOC]
        c2h = (fss[:, FREQ_IDX] * C2_SIGN[None, :]).T
        c1 = np.ascontiguousarray(np.concatenate([c1h, c1h], 0).astype(bf))
        c2 = np.ascontiguousarray(np.concatenate([c2h, c2h], 0).astype(bf))
        in_maps.append({"xT": xT, "wqT": wqT, "wkT": wkT, "wvT": wvT,
                        "woT": woT, "c1": c1, "c2": c2})

    res = None
    for attempt in range(3):
        try:
            res = run_bass_kernel_spmd(nc, in_maps, list(range(NCORE)))
            break
        except Exception:
            if attempt == 2:
                raise
            import time
            time.sleep(10)   # transient NRT device errors usually clear on retry
    LAST_RESULTS = res
    out = np.concatenate([res.results[c]["out"] for c in range(NCORE)], axis=0)
    return np.ascontiguousarray(out.astype(np.float32))

